# revision 7
# baseline (speedup 1.0000x reference)
"""Trainium2 8-core attention kernel (v4).

Problem: B=2, H=16, S=2048, D=64 dense attention, f32 I/O.
Sharding: B*H = 32 head-batches -> 4 heads per NeuronCore (embarrassingly
parallel, no collectives).

Per-core algorithm (transposed score space end-to-end):
  S^T[k, q] = K_dmaj . Q_dmaj      PE matmul, contraction d=64, ROW-TILED
                                   (two concurrent matmuls, row groups 0/64)
  P = exp(S^T / 8)                 hybrid exp: ScalarE ACT for 75/128 tiles,
                                   VectorE Schraudolph (f32->int16
                                   round(A*s+B) bitcast to bf16) for 53/128
  outT[d', q] = V'^T @ P           PE matmul, contraction k; V'=[V|ones] so
                                   row 64 = softmax denominator
  outT[:64] *= recip(den)          DVE recip from PSUM + gpsimd partition
                                   broadcast + DVE multiply; output stays
                                   [d, q], host transposes back (layout only)

Schedule (v4): uniform software pipeline at half-head granularity.
  half 0 of head h hosts PV chunks {2,3} of head h-1;
  half 1 of head h hosts PV chunks {0,1} of head h (these only need
  half-0 exps, which are complete). The last head's chunks {2,3} trail
  its half-1 exp stream at kt lag 4, shrinking the tail to a few MMs.
  QK pairs and PV matmuls are emitted in coarse groups ([4 QK | 8 PV])
  to minimize QK<->PV weight-switch transitions on the PE (~95ns each).

Host side only reshapes/transposes/casts (layout choices for sharding):
  qt, kt: [4, 128, 2048] bf16 (d on partitions, rows 64:128 duplicate 0:64)
  vp:     [4, 128, 16, 65] bf16 (k%128 on partitions, ones column appended)
  ot:     [4, 64, 2048] f32 (transposed; host transposes to [4, 2048, 64])
"""

import numpy as np
import ml_dtypes

import concourse.bass as bass
import concourse.tile as tile
from concourse import bacc, mybir
from concourse.bass_utils import run_bass_kernel_spmd

B, H, S, D = 2, 16, 2048, 64
NCORES = 8
HPC = (B * H) // NCORES  # heads per core = 4
P = 128
KT = S // P  # 16 k-tiles
SCALE = 1.0 / np.sqrt(D)  # 0.125

# Schraudolph bf16-exp constants: bits16 = round(A*s + B); bitcast -> bf16
SCH_A = float(P * np.log2(np.e) * SCALE)
SCH_B = float(P * 127 - 7.5)

# exp engine split: 76 tiles on ScalarE, 52 on VectorE (DVE also carries the
# normalize epilogue, and the last half carries the trailing epilogues), spread
# evenly through each half so neither engine gates the QK stream.
_DVE_SETS = {
    7: frozenset({1, 3, 5, 8, 10, 12, 14}),
    6: frozenset({1, 3, 6, 9, 11, 14}),
    4: frozenset({2, 6, 10, 13}),
}
# per (h, half) DVE tile counts
_DVE_COUNT = {
    (0, 0): 7, (0, 1): 7, (1, 0): 7, (1, 1): 7,
    (2, 0): 7, (2, 1): 7, (3, 0): 6, (3, 1): 4,
}


def is_dve_tile(kt_i, half, h):
    return kt_i in _DVE_SETS[_DVE_COUNT[(h, half)]]


f32 = mybir.dt.float32
bf16 = mybir.dt.bfloat16
i16 = mybir.dt.int16


def emit_loads(nc, pools, aps, h):
    qt, kt, vp, ot = aps
    qk_pool, v_pool, p_pool, epi_pool, ps_s, ps_o = pools
    qt_b = qk_pool.tile([P, S], bf16, tag="qt")
    kt_b = qk_pool.tile([P, S], bf16, tag="kt")
    if h == 0:
        # first head's QK deps are the critical path out of the preamble:
        # spread kt + first qt half across three queues
        nc.sync.dma_start(kt_b[:, :512], kt[h, :, :512])
        nc.scalar.dma_start(qt_b[:, :1024], qt[h, :, :1024])
        nc.gpsimd.dma_start(kt_b[:, 512:1024], kt[h, :, 512:1024])
        nc.sync.dma_start(kt_b[:, 1024:], kt[h, :, 1024:])
        nc.scalar.dma_start(qt_b[:, 1024:], qt[h, :, 1024:])
    else:
        nc.sync.dma_start(kt_b[:, : S // 2], kt[h, :, : S // 2])
        nc.sync.dma_start(qt_b[:, : S // 2], qt[h, :, : S // 2])
        nc.sync.dma_start(kt_b[:, S // 2 :], kt[h, :, S // 2 :])
        nc.sync.dma_start(qt_b[:, S // 2 :], qt[h, :, S // 2 :])
    v_b = v_pool.tile([P, KT, D + 1], bf16, tag="v")
    nc.sync.dma_start(v_b[:], vp[h])
    p_b = p_pool.tile([P, KT, S], bf16, tag="p")
    return qt_b, kt_b, v_b, p_b


def emit_qk_tile(nc, pools, half, kt_i, qt_b, kt_b, p_b, h):
    """One [128, 1024] score tile: row-tiled QK pair + exp."""
    qk_pool, v_pool, p_pool, epi_pool, ps_s, ps_o = pools
    q0 = half * 1024
    s_ps = ps_s.tile([P, 1024], f32, tag="s")
    nc.tensor.matmul(
        s_ps[:, 0:512],
        lhsT=kt_b[0:64, kt_i * P : (kt_i + 1) * P],
        rhs=qt_b[0:64, q0 : q0 + 512],
        start=True,
        stop=True,
        tile_position=(0, 0),
    )
    nc.tensor.matmul(
        s_ps[:, 512:1024],
        lhsT=kt_b[64:128, kt_i * P : (kt_i + 1) * P],
        rhs=qt_b[64:128, q0 + 512 : q0 + 1024],
        start=True,
        stop=True,
        tile_position=(64, 0),
    )
    dst = p_b[:, kt_i, q0 : q0 + 1024]
    if is_dve_tile(kt_i, half, h):
        nc.vector.tensor_scalar(
            dst.bitcast(i16),
            s_ps[:],
            SCH_A,
            SCH_B,
            mybir.AluOpType.mult,
            mybir.AluOpType.add,
        )
    else:
        nc.scalar.activation(
            dst, s_ps[:], mybir.ActivationFunctionType.Exp, scale=float(SCALE)
        )


class PVChunk:
    """One 512-wide q-chunk of a head's PV, fed matmul-by-matmul so the MMs
    interleave with the QK stream instead of starving the exp engines."""

    def __init__(self, h, p_b, v_b, qc):
        self.h, self.p_b, self.v_b, self.qc = h, p_b, v_b, qc
        self.o_ps = None
        self.k = 0

    def step(self, nc, pools, aps, n_mm):
        qt, kt, vp, ot = aps
        qk_pool, v_pool, p_pool, epi_pool, ps_s, ps_o = pools
        if self.o_ps is None:
            self.o_ps = ps_o.tile([P, 512], f32, tag="o")
        for _ in range(n_mm):
            if self.k >= KT:
                break
            nc.tensor.matmul(
                self.o_ps[: D + 1, :],
                lhsT=self.v_b[:, self.k, :],
                rhs=self.p_b[:, self.k, self.qc * 512 : (self.qc + 1) * 512],
                start=(self.k == 0),
                stop=(self.k == KT - 1),
                skip_group_check=True,
            )
            self.k += 1
        if self.k == KT:
            self.finish(nc, pools, aps)

    def finish(self, nc, pools, aps):
        qt, kt, vp, ot = aps
        qk_pool, v_pool, p_pool, epi_pool, ps_s, ps_o = pools
        o_ps = self.o_ps
        # row D of the PV psum is the softmax denominator (ones column of V').
        # NOTE: reciprocal_approx_fast is a custom DVE op and reads garbage
        # from PSUM on real HW (CoreSim accepts it) — copy to SBUF first.
        den = epi_pool.tile([1, 512], f32, tag="den")
        nc.vector.tensor_copy(den[:], o_ps[D : D + 1, :])
        rden = epi_pool.tile([1, 512], f32, tag="rden")
        nc.vector.reciprocal_approx_fast(rden[:], den[:])
        rden_bc = epi_pool.tile([D, 512], f32, tag="rbc")
        nc.gpsimd.partition_broadcast(rden_bc[:], rden[0:1, :])
        ot_sb = epi_pool.tile([D, 512], f32, tag="ot")
        nc.vector.tensor_mul(ot_sb[:], o_ps[:D, :], rden_bc[:])
        nc.sync.dma_start(
            ot[self.h, :, self.qc * 512 : (self.qc + 1) * 512], ot_sb[:]
        )
        self.k = KT + 1  # mark done


def build_nc():
    nc = bacc.Bacc("TRN2", target_bir_lowering=False, debug=False)
    qt = nc.dram_tensor("qt", [HPC, P, S], bf16, kind="ExternalInput").ap()
    kt = nc.dram_tensor("kt", [HPC, P, S], bf16, kind="ExternalInput").ap()
    vp = nc.dram_tensor("vp", [HPC, P, KT, D + 1], bf16, kind="ExternalInput").ap()
    ot = nc.dram_tensor("ot", [HPC, D, S], f32, kind="ExternalOutput").ap()
    aps = (qt, kt, vp, ot)

    with tile.TileContext(nc) as tc:
        with (
            tc.tile_pool(name="qk", bufs=2) as qk_pool,
            tc.tile_pool(name="v", bufs=2) as v_pool,
            tc.tile_pool(name="p", bufs=2) as p_pool,
            tc.tile_pool(name="epi", bufs=3) as epi_pool,
            tc.tile_pool(name="ps_s", bufs=3, space="PSUM") as ps_s,
            tc.tile_pool(name="ps_o", bufs=2, space="PSUM") as ps_o,
        ):
            pools = (qk_pool, v_pool, p_pool, epi_pool, ps_s, ps_o)

            # HAM warm-up: dummy matmuls during the NEFF preamble so the PE
            # clock is at 8/8 when the real stream starts (~3.4us needed).
            warm_w = qk_pool.tile([P, P], bf16, tag="warm")
            nc.gpsimd.memset(warm_w[:], 0.0)
            warm_ps = ps_o.tile([P, 512], f32, tag="o")
            for _ in range(16):
                nc.tensor.matmul(
                    warm_ps[:, :P], lhsT=warm_w[:], rhs=warm_w[:],
                    start=True, stop=True,
                )

            prev = None  # (p_b, v_b) of head h-1
            for h in range(HPC):
                qt_b, kt_b, v_b, p_b = emit_loads(nc, pools, aps, h)
                last = h == HPC - 1
                for half in range(2):
                    # PV chunks hosted by this half (fully-ready data only):
                    #   half 0: chunks {2,3} of head h-1
                    #   half 1: chunks {0,1} of head h (half-0 exps done)
                    if half == 0:
                        jobs = (
                            [PVChunk(h - 1, *prev, 2), PVChunk(h - 1, *prev, 3)]
                            if prev is not None
                            else []
                        )
                    else:
                        jobs = [PVChunk(h, p_b, v_b, 0), PVChunk(h, p_b, v_b, 1)]
                    trail = (
                        [PVChunk(h, p_b, v_b, 2), PVChunk(h, p_b, v_b, 3)]
                        if (last and half == 1)
                        else []
                    )
                    for kt_i in range(KT):
                        emit_qk_tile(nc, pools, half, kt_i, qt_b, kt_b, p_b, h)
                        if last and half == 1:
                            # front-load chunks {0,1} as whole bursts so their
                            # PSUM banks free early; trail chunks {2,3} behind
                            # the half-1 exp stream so the tail after the last
                            # exp is only a few MMs
                            if kt_i == 1:
                                jobs[0].step(nc, pools, aps, KT)
                            if kt_i == 5:
                                jobs[1].step(nc, pools, aps, KT)
                            if kt_i >= 5:
                                trail[0].step(nc, pools, aps, 1)
                            if kt_i >= 9:
                                trail[1].step(nc, pools, aps, 1)
                        elif jobs:
                            # uniform 2 PV MMs per QK tile: famine-free at
                            # half boundaries, matches the exp-paced QK cadence
                            jobs[kt_i % 2].step(nc, pools, aps, 2)
                        else:
                            # head 0 half 0 has no PV work: keep the PE duty
                            # cycle high with dummy matmuls so the HAM clock
                            # gate stays at 8/8 for the dense phase
                            nc.tensor.matmul(
                                warm_ps[:, :512], lhsT=warm_w[:],
                                rhs=kt_b[:, :512], start=True, stop=True,
                            )
                            nc.tensor.matmul(
                                warm_ps[:, :512], lhsT=warm_w[:],
                                rhs=kt_b[:, :512], start=True, stop=True,
                            )
                    for job in trail:
                        job.step(nc, pools, aps, KT)
                prev = (p_b, v_b)

    nc.compile()
    return nc


def shard_inputs(Q, K, V):
    """Full [B,H,S,D] f32 -> per-core input maps (layout + dtype choices)."""
    Qh = np.asarray(Q, dtype=np.float32).reshape(B * H, S, D)
    Kh = np.asarray(K, dtype=np.float32).reshape(B * H, S, D)
    Vh = np.asarray(V, dtype=np.float32).reshape(B * H, S, D)

    in_maps = []
    for c in range(NCORES):
        sl = slice(c * HPC, (c + 1) * HPC)
        qt = np.empty((HPC, P, S), dtype=ml_dtypes.bfloat16)
        kt = np.empty((HPC, P, S), dtype=ml_dtypes.bfloat16)
        qt[:, :D, :] = Qh[sl].transpose(0, 2, 1).astype(ml_dtypes.bfloat16)
        kt[:, :D, :] = Kh[sl].transpose(0, 2, 1).astype(ml_dtypes.bfloat16)
        qt[:, D:, :] = qt[:, :D, :]  # duplicate for row-group 64-127
        kt[:, D:, :] = kt[:, :D, :]
        vp = np.ones((HPC, S, D + 1), dtype=np.float32)
        vp[:, :, :D] = Vh[sl]
        # [h, (kt p), d] -> [h, p, kt, d']
        vp = (
            vp.reshape(HPC, KT, P, D + 1)
            .transpose(0, 2, 1, 3)
            .astype(ml_dtypes.bfloat16)
        )
        in_maps.append({"qt": np.ascontiguousarray(qt),
                        "kt": np.ascontiguousarray(kt),
                        "vp": np.ascontiguousarray(vp)})
    return in_maps


_NC_CACHE = None


def kernel(Q, K, V):
    global _NC_CACHE
    if _NC_CACHE is None:
        _NC_CACHE = build_nc()
    nc = _NC_CACHE
    in_maps = shard_inputs(Q, K, V)
    res = run_bass_kernel_spmd(nc, in_maps, core_ids=list(range(NCORES)))
    out = np.empty((B * H, S, D), dtype=np.float32)
    for c in range(NCORES):
        out[c * HPC : (c + 1) * HPC] = res.results[c]["ot"].transpose(0, 2, 1)
    return out.reshape(B, H, S, D)


if __name__ == "__main__":
    nc = build_nc()
    print("compiled OK")


# revision 8
# speedup vs baseline: 1.0560x; 1.0560x over previous
"""Trainium2 8-core attention kernel (v3).

Problem: B=2, H=16, S=2048, D=64 dense attention, f32 I/O.
Sharding: B*H = 32 head-batches -> 4 heads per NeuronCore (embarrassingly
parallel, no collectives).

Per-core algorithm (transposed score space end-to-end):
  S^T[k, q] = K_dmaj . Q_dmaj      PE matmul, contraction d=64, ROW-TILED
                                   (two concurrent matmuls, row groups 0/64)
  P = exp(S^T / 8)                 hybrid exp: ScalarE ACT for 85/128 tiles,
                                   VectorE Schraudolph (f32->int16
                                   round(A*s+B) bitcast to bf16) for 43/128
  outT[d', q] = V'^T @ P           PE matmul, contraction k; V'=[V|ones] so
                                   row 64 = softmax denominator
  outT[:64] /= den                 DVE recip_approx + gpsimd partition
                                   broadcast + DVE multiply; output stays
                                   [d, q]; the host transposes back (layout)

Host side only reshapes/transposes/casts (layout choices for sharding):
  qt, kt: [4, 128, 2048] bf16 (d on partitions, rows 64:128 duplicate 0:64)
  vp:     [4, 128, 16, 65] bf16 (k%128 on partitions, ones column appended)
  ot:     [4, 64, 2048] f32 (transposed; host transposes to [4, 2048, 64])
"""

import numpy as np
import ml_dtypes

import concourse.bass as bass
import concourse.tile as tile
from concourse import bacc, mybir
from concourse.bass_utils import run_bass_kernel_spmd

B, H, S, D = 2, 16, 2048, 64
NCORES = 8
HPC = (B * H) // NCORES  # heads per core = 4
P = 128
KT = S // P  # 16 k-tiles
SCALE = 1.0 / np.sqrt(D)  # 0.125

SCH_A = float(P * np.log2(np.e) * SCALE)
SCH_B = float(P * 127 - 7.5)

_DVE_H0 = {2, 5, 8, 11, 14}
_DVE_H1 = {1, 4, 7, 10, 12, 15}
_DVE_LAST = {1, 3, 5, 7, 9}


def is_dve_tile(kt_i, half, h):
    if h == HPC - 1 and half == 1:
        return kt_i in _DVE_LAST
    return kt_i in (_DVE_H0 if half == 0 else _DVE_H1)


f32 = mybir.dt.float32
bf16 = mybir.dt.bfloat16
i16 = mybir.dt.int16


def emit_loads(nc, pools, aps, h):
    qt, kt, vp, ot = aps
    qk_pool, v_pool, p_pool, epi_pool, ps_s, ps_o = pools
    qt_b = qk_pool.tile([P, S], bf16, tag="qt")
    kt_b = qk_pool.tile([P, S], bf16, tag="kt")
    if h == 0:
        nc.sync.dma_start(kt_b[:, :P], kt[h, :, :P])
        nc.scalar.dma_start(qt_b[:, :1024], qt[h, :, :1024])
        nc.gpsimd.dma_start(kt_b[:, P : S // 2], kt[h, :, P : S // 2])
        nc.sync.dma_start(kt_b[:, S // 2 :], kt[h, :, S // 2 :])
        nc.scalar.dma_start(qt_b[:, 1024:], qt[h, :, 1024:])
    else:
        nc.sync.dma_start(kt_b[:, : S // 2], kt[h, :, : S // 2])
        nc.sync.dma_start(qt_b[:, : S // 2], qt[h, :, : S // 2])
        nc.sync.dma_start(kt_b[:, S // 2 :], kt[h, :, S // 2 :])
        nc.sync.dma_start(qt_b[:, S // 2 :], qt[h, :, S // 2 :])
    v_b = v_pool.tile([P, KT, D + 1], bf16, tag="v")
    nc.sync.dma_start(v_b[:], vp[h])
    p_b = p_pool.tile([P, KT, S], bf16, tag="p")
    return qt_b, kt_b, v_b, p_b


def emit_qk_tile(nc, pools, half, kt_i, qt_b, kt_b, p_b, h=1):
    qk_pool, v_pool, p_pool, epi_pool, ps_s, ps_o = pools
    q0 = half * 1024
    s_ps = ps_s.tile([P, 1024], f32, tag="s")
    nc.tensor.matmul(
        s_ps[:, 0:512],
        lhsT=kt_b[0:64, kt_i * P : (kt_i + 1) * P],
        rhs=qt_b[0:64, q0 : q0 + 512],
        start=True,
        stop=True,
        tile_position=(0, 0),
    )
    nc.tensor.matmul(
        s_ps[:, 512:1024],
        lhsT=kt_b[64:128, kt_i * P : (kt_i + 1) * P],
        rhs=qt_b[64:128, q0 + 512 : q0 + 1024],
        start=True,
        stop=True,
        tile_position=(64, 0),
    )
    dst = p_b[:, kt_i, q0 : q0 + 1024]
    if is_dve_tile(kt_i, half, h):
        nc.vector.tensor_scalar(
            dst.bitcast(i16),
            s_ps[:],
            SCH_A,
            SCH_B,
            mybir.AluOpType.mult,
            mybir.AluOpType.add,
        )
    else:
        nc.scalar.activation(
            dst, s_ps[:], mybir.ActivationFunctionType.Exp, scale=float(SCALE)
        )


class PVChunk:
    def __init__(self, h, p_b, v_b, qc):
        self.h, self.p_b, self.v_b, self.qc = h, p_b, v_b, qc
        self.o_ps = None
        self.k = 0

    def step(self, nc, pools, aps, n_mm):
        qt, kt, vp, ot = aps
        qk_pool, v_pool, p_pool, epi_pool, ps_s, ps_o = pools
        if self.o_ps is None:
            self.o_ps = ps_o.tile([P, 512], f32, tag="o")
        for _ in range(n_mm):
            if self.k >= KT:
                return
            nc.tensor.matmul(
                self.o_ps[: D + 1, :],
                lhsT=self.v_b[:, self.k, :],
                rhs=self.p_b[:, self.k, self.qc * 512 : (self.qc + 1) * 512],
                start=(self.k == 0),
                stop=(self.k == KT - 1),
                skip_group_check=True,
            )
            self.k += 1
        if self.k >= KT:
            self.finish(nc, pools, aps)

    def finish(self, nc, pools, aps):
        qt, kt, vp, ot = aps
        qk_pool, v_pool, p_pool, epi_pool, ps_s, ps_o = pools
        o_ps = self.o_ps
        den = epi_pool.tile([1, 512], f32, tag="den")
        nc.vector.tensor_copy(den[:], o_ps[D : D + 1, :])
        rden = epi_pool.tile([1, 512], f32, tag="rden")
        nc.vector.reciprocal_approx_fast(rden[:], den[:])
        rden_bc = epi_pool.tile([D, 512], f32, tag="rbc")
        nc.gpsimd.partition_broadcast(rden_bc[:], rden[0:1, :])
        ot_sb = epi_pool.tile([D, 512], f32, tag="ot")
        nc.vector.tensor_mul(ot_sb[:], o_ps[:D, :], rden_bc[:])
        nc.sync.dma_start(
            ot[self.h, :, self.qc * 512 : (self.qc + 1) * 512], ot_sb[:]
        )
        self.k = KT + 1


def emit_pv_qc(nc, pools, aps, h, p_b, v_b, qc):
    ch = PVChunk(h, p_b, v_b, qc)
    ch.step(nc, pools, aps, KT)


def build_nc():
    nc = bacc.Bacc("TRN2", target_bir_lowering=False, debug=False)
    qt = nc.dram_tensor("qt", [HPC, P, S], bf16, kind="ExternalInput").ap()
    kt = nc.dram_tensor("kt", [HPC, P, S], bf16, kind="ExternalInput").ap()
    vp = nc.dram_tensor("vp", [HPC, P, KT, D + 1], bf16, kind="ExternalInput").ap()
    ot = nc.dram_tensor("ot", [HPC, D, S], f32, kind="ExternalOutput").ap()
    aps = (qt, kt, vp, ot)

    with tile.TileContext(nc) as tc:
        with (
            tc.tile_pool(name="qk", bufs=2) as qk_pool,
            tc.tile_pool(name="v", bufs=2) as v_pool,
            tc.tile_pool(name="p", bufs=2) as p_pool,
            tc.tile_pool(name="epi", bufs=3) as epi_pool,
            tc.tile_pool(name="ps_s", bufs=3, space="PSUM") as ps_s,
            tc.tile_pool(name="ps_o", bufs=2, space="PSUM") as ps_o,
        ):
            pools = (qk_pool, v_pool, p_pool, epi_pool, ps_s, ps_o)

            warm_w = qk_pool.tile([P, P], bf16, tag="warm")
            nc.gpsimd.memset(warm_w[:], 0.0)
            warm_ps = ps_o.tile([P, 512], f32, tag="o")
            for _ in range(30):
                nc.tensor.matmul(
                    warm_ps[:, :P], lhsT=warm_w[:], rhs=warm_w[:],
                    start=True, stop=True,
                )

            prev = None
            for h in range(HPC):
                qt_b, kt_b, v_b, p_b = emit_loads(nc, pools, aps, h)
                last = h == HPC - 1
                for half in range(2):
                    jobs = []
                    if prev is not None:
                        jobs.append(PVChunk(h - 1, *prev, 2 * half))
                        jobs.append(PVChunk(h - 1, *prev, 2 * half + 1))
                    if last and half == 1:
                        jobs.append(PVChunk(h, p_b, v_b, 0))
                        jobs.append(PVChunk(h, p_b, v_b, 1))
                    if len(jobs) == 2:
                        burst_at = {5: 0, 11: 1}
                    elif len(jobs) == 4:
                        burst_at = {5: 0, 8: 2, 11: 1, 14: 3}
                    else:
                        burst_at = {}
                    for kt_i in range(KT):
                        emit_qk_tile(
                            nc, pools, half, kt_i, qt_b, kt_b, p_b, h
                        )
                        if kt_i in burst_at:
                            jobs[burst_at[kt_i]].step(nc, pools, aps, KT)
                prev = (p_b, v_b)
            for qc in (2, 3):
                emit_pv_qc(nc, pools, aps, HPC - 1, *prev, qc)

    nc.compile()
    return nc


def shard_inputs(Q, K, V):
    Qh = np.asarray(Q, dtype=np.float32).reshape(B * H, S, D)
    Kh = np.asarray(K, dtype=np.float32).reshape(B * H, S, D)
    Vh = np.asarray(V, dtype=np.float32).reshape(B * H, S, D)

    in_maps = []
    for c in range(NCORES):
        sl = slice(c * HPC, (c + 1) * HPC)
        qt = np.empty((HPC, P, S), dtype=ml_dtypes.bfloat16)
        kt = np.empty((HPC, P, S), dtype=ml_dtypes.bfloat16)
        qt[:, :D, :] = Qh[sl].transpose(0, 2, 1).astype(ml_dtypes.bfloat16)
        kt[:, :D, :] = Kh[sl].transpose(0, 2, 1).astype(ml_dtypes.bfloat16)
        qt[:, D:, :] = qt[:, :D, :]
        kt[:, D:, :] = kt[:, :D, :]
        vp = np.ones((HPC, S, D + 1), dtype=np.float32)
        vp[:, :, :D] = Vh[sl]
        vp = (
            vp.reshape(HPC, KT, P, D + 1)
            .transpose(0, 2, 1, 3)
            .astype(ml_dtypes.bfloat16)
        )
        in_maps.append({"qt": np.ascontiguousarray(qt),
                        "kt": np.ascontiguousarray(kt),
                        "vp": np.ascontiguousarray(vp)})
    return in_maps


_NC_CACHE = None


def kernel(Q, K, V):
    global _NC_CACHE
    if _NC_CACHE is None:
        _NC_CACHE = build_nc()
    nc = _NC_CACHE
    in_maps = shard_inputs(Q, K, V)
    res = run_bass_kernel_spmd(nc, in_maps, core_ids=list(range(NCORES)))
    out = np.empty((B * H, S, D), dtype=np.float32)
    for c in range(NCORES):
        out[c * HPC : (c + 1) * HPC] = res.results[c]["ot"].transpose(0, 2, 1)
    return out.reshape(B, H, S, D)


if __name__ == "__main__":
    nc = build_nc()
    print("compiled OK")
